# revision 1
# baseline (speedup 1.0000x reference)
"""Trainium2 Bass kernel for nn_KTM_71339406786898.

Fused dual-input attention block (see reference.py), data-parallel over
batch B=8 across 8 NeuronCores, one batch element per core.

Per-core pipeline (flash-style, j-tiles of 128 on partitions):
  prologue: xcat=[x2;x3;1], xmul=[x2*x3;1] -> q,k (fp8e4, DoubleRow
  layout [8,2,HW], replicated at 4 PE row-band offsets), v^T per j-tile
  (fp8e4 stack [128, NJT, 65] = v2|v3|ones).
  main loop over 8 i-chunks of 512 columns:
    energy  E[j,i] (PE fp8 DoubleRow, K=16 folded to [8,2], 4 row bands)
    exp     e5m2 weights: ACT exp (bias-shifted) or DVE Schraudolph
            int8 bit-trick (engine split tuned by ESCHED)
    apply   acc[65,512] += [v|1]^T e  (PE fp8 DoubleRow, 2 j-tiles/mm)
    norm    r = bit-trick recip of acc[64]; broadcast (gpsimd);
            z = acc[0:64]*r + [x2;x3]  -> padded conv plane [64, 66*66]
    conv    9 matmuls/branch K=32 straight from plane (two concurrent
            32x32 PE tiles), relu+bn bias, fused (wo@w{2,3}_1) matmul,
            +bias (ACT), DMA out.  Runs one chunk behind attention.

Host folds: biases via ones rows, gamma into wv, bn_s into conv
weights, wo@w{2,3}_1 into one [64,32] matmul.
"""

import sys

import ml_dtypes
import numpy as np

for _p in ("/opt/trn_rl_repo", "/root/.axon_site/_ro/trn_rl_repo"):
    if _p not in sys.path:
        sys.path.append(_p)

import concourse.bass as bass
import concourse.mybir as mybir
import concourse.tile as tile
from concourse import bacc
from concourse.bass_utils import run_bass_kernel_spmd

B, C, H, W = 8, 32, 64, 64
CQ = C // 2
HW = H * W
NCORES = 8

IC = 512             # i-chunk width
NCH = HW // IC       # 8 chunks
JT = 128             # j-tile (partitions)
NJT = HW // JT       # 32 j-tiles
NGR = NJT // 2       # 16 granules of 2 j-tiles (DoubleRow pairs)
PW = W + 2           # padded conv width (66)
PHW = PW * (H + 2)   # padded conv plane
RPC = IC // W        # spatial rows per chunk (8)

F32 = mybir.dt.float32
F32R = mybir.dt.float32r
I32 = mybir.dt.int32
I8 = mybir.dt.int8
F8E4 = mybir.dt.float8e4
F8E5 = mybir.dt.float8e5
AF = mybir.ActivationFunctionType
ALU = mybir.AluOpType
DR = mybir.MatmulPerfMode.DoubleRow

# Schraudolph e5m2: i8 = E*4*log2(e) + SCH_C, bitcast -> 2^((i-60)/4)
# ~= e^E * 2^((SCH_C-60)/4).  ACT path matches via exp-input bias
# ln(2)*(SCH_C-60)/4.
SCH_C = 63.0
SCH_SLOPE = float(4.0 / np.log(2.0))
ACT_BIAS = float(np.log(2.0) * (SCH_C - 60.0) / 4.0)
RECIP_MAGIC = 0x7EF312AC

# exp engine schedule per chunk: one entry per granule, 'A'=ACT 'D'=DVE
ESCHED = "ADADADADADADADAD"
assert len(ESCHED) == NGR


def _r(ap):
    return ap.bitcast(F32R)


def build_program():
    nc = bacc.Bacc("TRN2", target_bir_lowering=False, debug=False)

    x2d = nc.dram_tensor("x2", [C, HW], F32R, kind="ExternalInput").ap()
    x3d = nc.dram_tensor("x3", [C, HW], F32R, kind="ExternalInput").ap()
    onesd = nc.dram_tensor("ones", [1, HW], F32R, kind="ExternalInput").ap()
    wq8d = nc.dram_tensor("wq8", [C * 2 + 1, CQ], F32R, kind="ExternalInput").ap()
    wk8d = nc.dram_tensor("wk8", [C + 1, CQ], F32R, kind="ExternalInput").ap()
    wvad = nc.dram_tensor("wva", [C * 2 + 1, 2 * C + 2], F32R, kind="ExternalInput").ap()
    wcsd = nc.dram_tensor("wcs", [3 * C, 6 * C], F32R, kind="ExternalInput").ap()
    bbd = nc.dram_tensor("bb", [2 * C, 1], F32, kind="ExternalInput").ap()
    wabd = nc.dram_tensor("wab", [C, 2 * C], F32R, kind="ExternalInput").ap()
    bfind = nc.dram_tensor("bfin", [C, 1], F32, kind="ExternalInput").ap()
    outd = nc.dram_tensor("out", [C, HW], F32, kind="ExternalOutput").ap()

    with tile.TileContext(nc) as tc:
        _emit(nc, tc, x2d, x3d, onesd, wq8d, wk8d, wvad, wcsd, bbd,
              wabd, bfind, outd)
    nc.compile()
    return nc


def _emit(nc, tc, x2d, x3d, onesd, wq8d, wk8d, wvad, wcsd, bbd,
          wabd, bfind, outd):
    from contextlib import ExitStack

    ctx = ExitStack()
    with ctx:
        consts = ctx.enter_context(tc.tile_pool(name="consts", bufs=1))
        xp = ctx.enter_context(tc.tile_pool(name="xp", bufs=1))
        qk = ctx.enter_context(tc.tile_pool(name="qk", bufs=1))
        vs = ctx.enter_context(tc.tile_pool(name="vs", bufs=1))
        es = ctx.enter_context(tc.tile_pool(name="es", bufs=20))
        zs = ctx.enter_context(tc.tile_pool(name="zs", bufs=2))
        outp = ctx.enter_context(tc.tile_pool(name="outp", bufs=2))
        zrp = ctx.enter_context(tc.tile_pool(name="zrp", bufs=1))
        ep = ctx.enter_context(tc.tile_pool(name="ep", bufs=2, space="PSUM"))
        accp = ctx.enter_context(tc.tile_pool(name="accp", bufs=2, space="PSUM"))
        cvp = ctx.enter_context(tc.tile_pool(name="cvp", bufs=2, space="PSUM"))

        # ---- constant loads -------------------------------------------------
        wq8 = consts.tile([C * 2 + 1, CQ], F32R, tag="wq8")
        nc.sync.dma_start(out=wq8[:], in_=wq8d)
        wk8 = consts.tile([C + 1, CQ], F32R, tag="wk8")
        nc.sync.dma_start(out=wk8[:], in_=wk8d)
        wva = consts.tile([C * 2 + 1, 2 * C + 2], F32R, tag="wva")
        nc.sync.dma_start(out=wva[:], in_=wvad)
        wcs = consts.tile([3 * C, 6 * C], F32R, tag="wcs")
        nc.sync.dma_start(out=wcs[:], in_=wcsd)
        bb2 = consts.tile([2 * C, 1], F32, tag="bb")
        nc.sync.dma_start(out=bb2[:], in_=bbd)
        wab2 = consts.tile([C, 2 * C], F32R, tag="wab")
        nc.sync.dma_start(out=wab2[:], in_=wabd)
        bfin = consts.tile([C, 1], F32, tag="bfin")
        nc.sync.dma_start(out=bfin[:], in_=bfind)
        ebias = consts.tile([JT, 1], F32, tag="ebias")
        nc.gpsimd.memset(ebias[:], ACT_BIAS)

        # ---- inputs ---------------------------------------------------------
        xcat = xp.tile([2 * C + 1, HW], F32R, tag="xcat")
        nc.sync.dma_start(out=xcat[0:C, :], in_=x2d)
        nc.sync.dma_start(out=xcat[C:2 * C, :], in_=x3d)
        nc.sync.dma_start(out=xcat[2 * C:2 * C + 1, :], in_=onesd)
        xmulc = xp.tile([C + 1, HW], F32R, tag="xmulc")
        nc.sync.dma_start(out=xmulc[C:C + 1, :], in_=onesd)
        # second copy of x3 at base partition 0 (HW requires equal base
        # partitions for SBUF*SBUF tensor_tensor inputs)
        x3b = xp.tile([C, HW], F32, tag="x3b")
        nc.sync.dma_start(out=x3b[:], in_=x3d.bitcast(F32))

        # ---- q/k projections into fp8 layout --------------------------------
        # q8/k8: [16 channels (+ replicas at 32/64/96), HW] fp8e4
        q8 = qk.tile([112, HW], F8E4, tag="q8")
        k8 = qk.tile([112, HW], F8E4, tag="k8")

        for blk in range(NCH):
            sl = slice(blk * IC, (blk + 1) * IC)
            if blk % 4 == 3:
                nc.gpsimd.tensor_mul(xmulc[0:C, sl],
                                     xcat[0:C, sl].bitcast(F32),
                                     x3b[:, sl])
            else:
                nc.vector.tensor_mul(xmulc[0:C, sl],
                                     xcat[0:C, sl].bitcast(F32),
                                     x3b[:, sl])
            pp = ep.tile([JT, 2 * IC], F32, tag="e")
            nc.tensor.matmul(pp[0:CQ, 0:IC], wk8[:], xmulc[:, sl],
                             start=True, stop=True)
            nc.tensor.matmul(pp[0:CQ, IC:2 * IC], wq8[:], xcat[:, sl],
                             start=True, stop=True)
            nc.scalar.activation(k8[0:CQ, sl], pp[0:CQ, 0:IC], AF.Copy)
            nc.vector.tensor_copy(out=q8[0:CQ, sl], in_=pp[0:CQ, IC:2 * IC])

        for rg in (1, 2, 3):
            eng = (nc.sync, nc.gpsimd, nc.scalar)[rg - 1]
            eng.dma_start(out=k8[32 * rg:32 * rg + CQ, :], in_=k8[0:CQ, :])
            eng.dma_start(out=q8[32 * rg:32 * rg + CQ, :], in_=q8[0:CQ, :])

        # ---- v^T stack: vstk[j_local, jt, c] = v2|v3|ones (fp8e4) ----------
        VM = 2 * C + 2   # 66 v columns (v2|v3|ones|pad)
        VST = 80         # j-tile stride in vstk (%16==0 for dual-fp8 LW)
        vstk = vs.tile([JT, NJT, VST], F8E4, tag="vstk")
        nc.gpsimd.memset(vstk[:, :, VM:VST], 0.0)
        VPB = 7  # v-tiles per psum half-bank
        jt = 0
        while jt < NJT:
            vt = ep.tile([JT, 2 * IC], F32, tag="e")
            n_here = min(2 * VPB, NJT - jt)
            for t in range(n_here):
                off = (t // VPB) * IC + (t % VPB) * VM
                nc.tensor.matmul(
                    vt[:, off:off + VM],
                    xcat[:, (jt + t) * JT:(jt + t + 1) * JT],
                    wva[:],
                    start=True, stop=True,
                )
            for half in range(2):
                cnt = min(VPB, n_here - half * VPB)
                if cnt <= 0:
                    break
                nc.scalar.activation(
                    vstk[:, jt + half * VPB:jt + half * VPB + cnt, 0:VM],
                    vt[:, half * IC:half * IC + cnt * VM]
                    .rearrange("p (t c) -> p t c", c=VM),
                    AF.Copy,
                )
            jt += n_here

        # ---- padded conv plane: z2 rows 0-31, z3 rows 32-63 ----------------
        z23p = zs.tile([2 * C, PHW], F32R, tag="z23p")
        z3d_ = z23p.rearrange("p (h w) -> p h w", h=H + 2, w=PW)
        nc.gpsimd.memset(z3d_[:, 0:1, :].bitcast(F32), 0.0)
        nc.gpsimd.memset(z3d_[:, H + 1:H + 2, :].bitcast(F32), 0.0)
        nc.gpsimd.memset(z3d_[:, 1:H + 1, 0:1].bitcast(F32), 0.0)
        nc.gpsimd.memset(z3d_[:, 1:H + 1, PW - 1:PW].bitcast(F32), 0.0)

        # ---- zR staging: 3 dy-shifted copies of the z plane rows -----------
        # zr[32*dy + ci, buf, t] = z23p[ci(+C), 8n*66 + dy*66 + t]
        ZRL = 544
        zr2 = zrp.tile([3 * C, 2, ZRL], F32R, tag="zr2")
        zr3 = zrp.tile([3 * C, 2, ZRL], F32R, tag="zr3")

        def emit_zr(n, gs):
            src0 = RPC * n * PW
            for g in gs:
                eng = nc.sync if g % 2 == 0 else nc.gpsimd
                eng.dma_start(
                    out=zr2[32 * g:32 * g + C, n % 2, 0:RPC * PW],
                    in_=z23p[0:C, src0 + g * PW:src0 + g * PW + RPC * PW])
                eng.dma_start(
                    out=zr3[32 * g:32 * g + C, n % 2, 0:RPC * PW],
                    in_=z23p[C:2 * C, src0 + g * PW:src0 + g * PW + RPC * PW])

        # ---- conv (runs two chunks behind the attention loop) --------------
        def conv_chunk(n):
            wb = n % 2
            cps2 = cvp.tile([C, IC], F32, tag="cv")
            cps3 = cvp.tile([C, IC], F32, tag="cv")
            for br, (zr, cp) in enumerate(((zr2, cps2), (zr3, cps3))):
                for dx in range(3):
                    nc.tensor.matmul(
                        cp[:],
                        wcs[:, (br * 3 + dx) * C:(br * 3 + dx + 1) * C],
                        zr[:, wb, dx:dx + RPC * PW]
                        .rearrange("p (r w) -> p r w", r=RPC, w=PW)[:, :, 0:W],
                        start=(dx == 0), stop=(dx == 2),
                    )
            rstk = outp.tile([C, 2, IC], F32R, tag="rstk")
            for br, cp in enumerate((cps2, cps3)):
                nc.vector.tensor_scalar(
                    out=rstk[:, br, :],
                    in0=cp[:],
                    scalar1=bb2[br * C:(br + 1) * C, 0:1], scalar2=0.0,
                    op0=ALU.add, op1=ALU.max,
                )
            op = cvp.tile([C, IC], F32, tag="cv")
            for br in range(2):
                nc.tensor.matmul(op[:], wab2[:, br * C:(br + 1) * C],
                                 rstk[:, br, :],
                                 start=(br == 0), stop=(br == 1))
            ob = outp.tile([C, IC], F32, tag="ob")
            nc.scalar.activation(ob[:], op[:], AF.Identity,
                                 bias=bfin[:, 0:1])
            nc.sync.dma_start(out=outd[:, n * IC:(n + 1) * IC], in_=ob[:])

        # ---- main attention loop (phase-batched PE stream) -----------------
        for ic in range(NCH):
            i0 = ic * IC
            acc = accp.tile([2 * C + 2, IC], F32, tag="acc")
            esq = []
            # phase 1: energies back-to-back, exp drains on ACT/DVE
            for g in range(NGR):
                e_ps = ep.tile([JT, 2 * IC], F32, tag="e")
                for t in range(2):
                    jt_ = 2 * g + t
                    b0 = 32 * (jt_ % 4)
                    nc.tensor.matmul(
                        e_ps[:, t * IC:(t + 1) * IC],
                        k8[b0:b0 + CQ, jt_ * JT:(jt_ + 1) * JT],
                        q8[b0:b0 + CQ, i0:i0 + IC],
                        start=True, stop=True,
                        tile_position=(b0, 0),
                    )
                e_sb = es.tile([JT, 2, IC], F8E5, tag="esb")
                if ESCHED[g] == 'A':
                    nc.scalar.activation(
                        e_sb[:, :, :],
                        e_ps[:].rearrange("p (t n) -> p t n", t=2),
                        AF.Exp, bias=ebias[:, 0:1],
                    )
                else:
                    nc.vector.tensor_scalar(
                        out=e_sb.bitcast(I8)[:, :, :],
                        in0=e_ps[:].rearrange("p (t n) -> p t n", t=2),
                        scalar1=SCH_SLOPE, scalar2=SCH_C,
                        op0=ALU.mult, op1=ALU.add,
                    )
                esq.append(e_sb)
            # phase 2: applies back-to-back
            for g in range(NGR):
                nc.tensor.matmul(
                    acc[:],
                    vstk[:, 2 * g:2 * g + 2, 0:VM],
                    esq[g][:, :, :],
                    start=(g == 0), stop=(g == NGR - 1),
                    perf_mode=DR,
                )
            # phase 3: conv of chunk ic-2
            if ic >= 2:
                conv_chunk(ic - 2)

            # normalize + residual into conv plane
            r_sb = zs.tile([1, IC], I32, tag="r_sb")
            nc.vector.tensor_scalar(
                out=r_sb[:], in0=acc[2 * C:2 * C + 1, :].bitcast(I32),
                scalar1=-1, scalar2=RECIP_MAGIC,
                op0=ALU.mult, op1=ALU.add,
            )
            rbc = zs.tile([2 * C, IC], F32, tag="rbc")
            nc.gpsimd.partition_broadcast(rbc[:], r_sb.bitcast(F32)[:])
            z23t = zs.tile([2 * C, IC], F32, tag="z23t")
            nc.vector.tensor_mul(z23t[:], acc[0:2 * C, :], rbc[:])
            r0 = RPC * ic
            for br in range(2):
                nc.vector.tensor_add(
                    z3d_[br * C:(br + 1) * C, 1 + r0:1 + r0 + RPC, 1:1 + W],
                    z23t[br * C:(br + 1) * C, :]
                    .rearrange("p (a b) -> p a b", a=RPC, b=W),
                    xcat[br * C:(br + 1) * C, i0:i0 + IC].bitcast(F32)
                    .rearrange("p (a b) -> p a b", a=RPC, b=W),
                )
            emit_zr(ic, (0, 1))
            if ic >= 1:
                emit_zr(ic - 1, (2,))
        emit_zr(NCH - 1, (2,))
        conv_chunk(NCH - 2)
        conv_chunk(NCH - 1)


def prepare_params(wq, bq, wk, bk, wv2, bv2, wv3, bv3, gamma2, gamma3,
                   w2_3, bn2_s, bn2_b, w2_1, b2_1,
                   w3_3, bn3_s, bn3_b, w3_1, b3_1, wo, bo):
    f = np.float32
    wq, bq, wk, bk = (np.asarray(a, f) for a in (wq, bq, wk, bk))
    wv2, bv2, wv3, bv3 = (np.asarray(a, f) for a in (wv2, bv2, wv3, bv3))
    w2_3, bn2_s, bn2_b = (np.asarray(a, f) for a in (w2_3, bn2_s, bn2_b))
    w3_3, bn3_s, bn3_b = (np.asarray(a, f) for a in (w3_3, bn3_s, bn3_b))
    w2_1, b2_1, w3_1, b3_1 = (np.asarray(a, f) for a in (w2_1, b2_1, w3_1, b3_1))
    wo, bo = np.asarray(wo, f), np.asarray(bo, f)
    g2 = f(np.asarray(gamma2).reshape(-1)[0])
    g3 = f(np.asarray(gamma3).reshape(-1)[0])

    wq8 = np.zeros((2 * C + 1, CQ), f)
    wq8[0:C] = wq.T
    wq8[C:2 * C] = wq.T
    wq8[2 * C] = bq
    wk8 = np.zeros((C + 1, CQ), f)
    wk8[0:C] = wk.T
    wk8[C] = bk
    # column j of wq8/wk8 = channel j with j = h*8+p <-> c = t*8+p: identity
    # (the two 8-wide halves are channels 0-7 and 8-15)

    wva = np.zeros((2 * C + 1, 2 * C + 2), f)
    wva[0:C, 0:C] = wv2.T * g2
    wva[C:2 * C, C:2 * C] = wv3.T * g3
    wva[2 * C, 0:C] = bv2 * g2
    wva[2 * C, C:2 * C] = bv3 * g3
    wva[2 * C, 2 * C] = 1.0

    wcs = np.zeros((3 * C, 6 * C), f)
    for (br, w3x3, bns) in ((0, w2_3, bn2_s), (1, w3_3, bn3_s)):
        ws = w3x3 * bns[:, None, None, None]  # [o, ci, dy, dx]
        for dy in range(3):
            for dx in range(3):
                wcs[C * dy:C * (dy + 1),
                    (br * 3 + dx) * C:(br * 3 + dx + 1) * C] = \
                    ws[:, :, dy, dx].T

    bb = np.concatenate([bn2_b, bn3_b]).reshape(2 * C, 1).astype(f)
    wab = np.zeros((C, 2 * C), f)
    wab[:, 0:C] = (wo @ w2_1).T
    wab[:, C:] = (wo @ w3_1).T
    bfin = (wo @ (b2_1 + b3_1) + bo).astype(f).reshape(C, 1)

    return {
        "wq8": wq8, "wk8": wk8, "wva": wva, "wcs": wcs, "bb": bb,
        "wab": wab, "bfin": bfin,
        "ones": np.ones((1, HW), f),
    }


_CACHED = {}


def _get_program():
    if "nc" not in _CACHED:
        _CACHED["nc"] = build_program()
    return _CACHED["nc"]


def make_in_maps(x2, x3, params):
    x2 = np.ascontiguousarray(np.asarray(x2, np.float32).reshape(B, C, HW))
    x3 = np.ascontiguousarray(np.asarray(x3, np.float32).reshape(B, C, HW))
    return [
        {"x2": x2[b], "x3": x3[b], **params}
        for b in range(NCORES)
    ]


def kernel(x2, x3, **kw):
    params = prepare_params(**kw)
    nc = _get_program()
    in_maps = make_in_maps(x2, x3, params)
    res = run_bass_kernel_spmd(nc, in_maps, list(range(NCORES)))
    out = np.stack([res.results[b]["out"].reshape(C, H, W)
                    for b in range(NCORES)])
    return out.astype(np.float32)


def _ensure_ntff_hook():
    import contextlib
    import ctypes
    import types

    if "antenv.axon_hooks" in sys.modules:
        return
    so_path = "/opt/axon/libaxon_pjrt.so"
    lib = ctypes.CDLL(so_path)
    lib.axon_start_nrt_profile.argtypes = [
        ctypes.POINTER(ctypes.c_int64), ctypes.c_size_t]
    lib.axon_start_nrt_profile.restype = ctypes.c_int64
    lib.axon_stop_nrt_profile.argtypes = [ctypes.c_char_p]
    lib.axon_stop_nrt_profile.restype = ctypes.c_int64

    @contextlib.contextmanager
    def _hook(output_dir, device_ids):
        import jax
        jax.devices()
        if device_ids:
            ids = (ctypes.c_int64 * len(device_ids))(*device_ids)
            rc = lib.axon_start_nrt_profile(ids, len(device_ids))
        else:
            rc = lib.axon_start_nrt_profile(None, 0)
        if rc != 0:
            raise RuntimeError(f"axon_start_nrt_profile rc={rc}")
        try:
            yield
        finally:
            n = lib.axon_stop_nrt_profile(str(output_dir).encode())
            if n < 0:
                raise RuntimeError(f"axon_stop_nrt_profile rc={n}")
            if n == 0:
                print("WARNING: NTFF capture wrote 0 files")

    mod = types.ModuleType("antenv.axon_hooks")
    mod.get_axon_ntff_profile_hook = lambda: _hook
    mod.set_axon_ntff_profile_hook = lambda h: None
    sys.modules["antenv.axon_hooks"] = mod


def run_traced(x2, x3, trace_cores=None, **kw):
    _ensure_ntff_hook()
    params = prepare_params(**kw)
    nc = _get_program()
    in_maps = make_in_maps(x2, x3, params)
    res = run_bass_kernel_spmd(nc, in_maps, list(range(NCORES)),
                               trace=True, trace_cores=trace_cores)
    out = np.stack([res.results[b]["out"].reshape(C, H, W)
                    for b in range(NCORES)])
    return out.astype(np.float32), res

